# revision 8
# baseline (speedup 1.0000x reference)
"""3-layer GAT on 8 trn2 NeuronCores.

Strategy (graph/data parallel per sharding hint):
  - Nodes are assigned to 8 cores x 49 blocks x 128 slots (degree-balanced
    LPT bin packing) -> permuted node order; "table row" = block*128 + slot.
  - Per layer: each core transforms its own node shard with
    rhs = [W | W@as | W@ad] (alpha terms folded into the matmul), writes a
    bf16 table shard [6272, EL], chunked AllGather -> full table on every
    core (chunks overlap with the previous layer's aggregation).
  - Aggregation: per dst-block of 128 nodes, edges (dst-sorted) are packed
    into 128-edge tiles; a dma_gather fetches table rows for the tile's
    sources; a one-hot "scatter matrix" matmul accumulates the s_e-weighted
    features AND the softmax denominator (s appended as extra rhs columns)
    into PSUM.  (Softmax max-shift is skipped: logits are O(1) so exp is
    safe, and the result is mathematically identical.)
  - int16 gather indices: table split into lo rows [0,32768) and hi rows
    [17408,50176); per-block edges are balanced between the (overlapping)
    windows so each side fits 9 tiles of 128.  Pad slots get index -1 so
    the Q7 descriptor generator strips them (trailing negatives).
  - dma_gather calls rotate across 4 SWDGE queues so descriptor generation
    parallelizes over the Q7 core pairs.
  - Transform of layer L+1 for block b runs right after block b's layer-L
    epilogue (per-block hT temp, no big hT buffer); shard chunks AllGather
    while later blocks still aggregate.
  - Layer 2 output is column-summed per core (masked for pad slots); the
    final mean + linear head run on host.
"""

import os
import numpy as np
import ml_dtypes

# ---------------- problem constants (must match reference) ----------------
N = 50000
E = 800000
IN_C = 128
HID = 64
HEADS = 4
OUT_C = 64
F1 = HEADS * HID  # 256

# ---------------- sharding geometry ----------------
NCORES = 8
NB = 49           # dst blocks per core
BS = 128          # dst slots per block
NPC = NB * BS     # 6272 nodes per core
RTOT = NCORES * NPC  # 50176 table rows
TL = 9            # tiles per kind (lo/hi)
TL2 = 2 * TL      # tiles per block (both kinds)
KE = TL * 128     # 1152 edge slots per (block, kind)
LO_LIM = 32768    # lo window rows [0, LO_LIM)
HI_BASE = 17408   # hi window rows [HI_BASE, HI_BASE+32768)
NKCOLS = KE // 16  # 72 idx columns per (block, kind)

TB_NP = ml_dtypes.bfloat16
EL01 = 384     # table elems/row layer0/1 (256 h + 4 as + 4 ad + pad), bf16
EL2 = 128      # table elems/row layer2 (64 h + 1 as + 1 ad + pad), bf16

NCHUNK = 4     # allgather chunks per layer
CHUNK_BOUNDS = [0, 13, 26, 38, 49]


def _chunk_of(b):
    for k in range(NCHUNK):
        if b < CHUNK_BOUNDS[k + 1]:
            return k
    raise ValueError(b)


def _rowbase(c, b):
    """Table row of (core c, block b, slot 0) in chunk-major table layout
    [chunk, core, blocks_in_chunk, slot] — keeps each chunk's AllGather
    output contiguous."""
    k = _chunk_of(b)
    base = 8 * BS * CHUNK_BOUNDS[k]
    ck = (CHUNK_BOUNDS[k + 1] - CHUNK_BOUNDS[k]) * BS
    return base + c * ck + (b - CHUNK_BOUNDS[k]) * BS


# ---------------- host preprocessing ----------------

def preprocess(edge_index):
    """Node->(core,block,slot) assignment and per-core edge tile arrays.

    Returns dict with:
      row:   [N] table row of each node
      xperm: [RTOT] node id occupying each table row (-1 for pad slots)
      idx16: [NCORES,128,NB*2*NKCOLS] int16 wrapped gather indices (-1 pad)
      dstc:  [NCORES,128,NB*2*TL] bf16 dst_local per edge slot (col layout,
             -1 pad)
      dstr:  [NCORES,128,KE] bf16 dst_local (row layout; partition=blk*2+kind)
      maskc: [NCORES,128,NB] bf16 1.0 for real-node slots
    """
    import heapq

    src = np.concatenate([np.asarray(edge_index[0]), np.arange(N, dtype=np.int64)])
    dst = np.concatenate([np.asarray(edge_index[1]), np.arange(N, dtype=np.int64)])
    deg = np.bincount(dst, minlength=N)

    nblocks = NCORES * NB
    order = np.argsort(-deg, kind="stable")
    heap = [(0, b) for b in range(nblocks)]
    heapq.heapify(heap)
    slots_used = np.zeros(nblocks, np.int64)
    node_block = np.empty(N, np.int64)
    node_slot = np.empty(N, np.int64)
    for n in order:
        while True:
            load, b = heapq.heappop(heap)
            if slots_used[b] < BS:
                break
        node_block[n] = b
        node_slot[n] = slots_used[b]
        slots_used[b] += 1
        heapq.heappush(heap, (load + int(deg[n]), b))
        # blocks that were full stay out of the heap

    # table row per node: chunk-major layout (see _rowbase)
    rowbase_cb = np.empty(nblocks, np.int64)
    for b in range(nblocks):
        c, bl = divmod(b, NB)
        rowbase_cb[b] = _rowbase(c, bl)
    row = rowbase_cb[node_block] + node_slot

    xperm = np.full(RTOT, -1, np.int64)
    xperm[row] = np.arange(N)

    erow = row[src]          # gather row per edge
    eblk = node_block[dst]   # destination block per edge
    eslot = node_slot[dst]   # dst_local per edge

    idx16 = np.zeros((NCORES, 128, NB * 2 * NKCOLS), np.int16)
    dstc = np.full((NCORES, 128, NB * 2 * TL), -1.0, TB_NP)
    dstr = np.zeros((NCORES, 128, KE), TB_NP)
    maskc = np.zeros((NCORES, 128, NB), TB_NP)

    order_e = np.argsort(eblk, kind="stable")
    bounds = np.searchsorted(eblk[order_e], np.arange(nblocks + 1))

    for b in range(nblocks):
        c, bl = divmod(b, NB)
        es = order_e[bounds[b]:bounds[b + 1]]
        r_ = erow[es]
        dl = eslot[es]
        lo_f = r_ < HI_BASE
        hi_f = r_ >= LO_LIM
        flex = ~lo_f & ~hi_f
        n_lo = int(lo_f.sum())
        n_hi = int(hi_f.sum())
        n_fx = int(flex.sum())
        tot = n_lo + n_hi + n_fx
        assert tot <= 2 * KE, f"block {b} has {tot} edges > {2*KE}"
        # send flex edges to lo until lo reaches ceil(tot/2) (capped at KE)
        add_lo = min(n_fx, max(0, min(KE, (tot + 1) // 2) - n_lo))
        if n_hi + (n_fx - add_lo) > KE:
            add_lo = n_fx - (KE - n_hi)
        assert 0 <= add_lo <= n_fx
        fx_idx = np.nonzero(flex)[0]
        sel_lo = np.zeros(len(es), bool)
        sel_lo[lo_f] = True
        sel_lo[fx_idx[:add_lo]] = True
        sel_hi = ~sel_lo
        assert sel_lo.sum() <= KE and sel_hi.sum() <= KE, (
            b, sel_lo.sum(), sel_hi.sum())

        for kind, sel, base in ((0, sel_lo, 0), (1, sel_hi, HI_BASE)):
            rr = r_[sel]
            dd = dl[sel]
            o = np.argsort(rr, kind="stable")  # DMA locality
            rr = rr[o]
            dd = dd[o]
            k = len(rr)
            rel = np.full(KE, -1, np.int64)   # -1 pads: Q7 strips trailing
            rel[:k] = rr - base
            dloc = np.full(KE, -1.0, np.float32)
            dloc[:k] = dd.astype(np.float32)
            assert k == 0 or (rel[:k].min() >= 0 and rel[:k].max() < 32768)
            # wrapped idx: index i -> [i % 16, i // 16]
            w = rel.reshape(NKCOLS, 16).T.astype(np.int16)  # [16, NKCOLS]
            cbase = (bl * 2 + kind) * NKCOLS
            idx16[c, :, cbase:cbase + NKCOLS] = np.tile(w, (8, 1))
            # col layout: col bl*2*TL + kind*TL + t, partition p = edge t*128+p
            tcol = bl * 2 * TL + kind * TL
            dstc[c, :, tcol:tcol + TL] = dloc.reshape(TL, 128).T.astype(TB_NP)
            # row layout: partition bl*2+kind
            dstr[c, bl * 2 + kind, :] = dloc.astype(TB_NP)

        # mask of real slots
        used = slots_used[b]
        maskc[c, :used, bl] = 1.0

    return dict(row=row, xperm=xperm, idx16=idx16, dstc=dstc, dstr=dstr,
                maskc=maskc, deg=deg, node_block=node_block,
                node_slot=node_slot)


def host_weights(inputs):
    """Extended weight matrices with folded attention vectors."""
    def ext(W, a_s, a_d, heads):
        # Was[k, h] = sum_c W[k, h*HID+c] * a_s[h, c]
        Wh = W.reshape(W.shape[0], heads, HID)
        Was = np.einsum("khc,hc->kh", Wh, a_s)
        Wad = np.einsum("khc,hc->kh", Wh, a_d)
        return np.concatenate([W, Was, Wad], axis=1).astype(np.float32)

    W0e = ext(np.asarray(inputs["W0"], np.float32),
              np.asarray(inputs["a0s"], np.float32),
              np.asarray(inputs["a0d"], np.float32), HEADS)      # [128, 264]
    W1e = ext(np.asarray(inputs["W1"], np.float32),
              np.asarray(inputs["a1s"], np.float32),
              np.asarray(inputs["a1d"], np.float32), HEADS)      # [256, 264]
    W2e = ext(np.asarray(inputs["W2"], np.float32),
              np.asarray(inputs["a2s"], np.float32),
              np.asarray(inputs["a2d"], np.float32), 1)          # [256, 66]
    return W0e, W1e, W2e


def build_core_inputs(inputs, pp):
    """Per-core in_maps for run_bass_kernel_spmd."""
    x = np.asarray(inputs["x"], np.float32)
    W0e, W1e, W2e = host_weights(inputs)
    b0 = np.asarray(inputs["b0"], np.float32)
    b1 = np.asarray(inputs["b1"], np.float32)
    b2 = np.asarray(inputs["b2"], np.float32)

    iota_row = np.tile(np.arange(128, dtype=np.float32), (128, 1)).astype(TB_NP)
    iota_col = np.arange(128, dtype=np.float32).reshape(128, 1).astype(TB_NP)
    ones1 = np.ones((1, 128), TB_NP)
    ident = np.eye(128, dtype=np.float32).astype(TB_NP)

    consts = dict(
        w0e=W0e.astype(TB_NP),                                  # [128, 264]
        w1e=W1e.reshape(2, 128, F1 + 2 * HEADS).astype(TB_NP),  # [2, 128, 264]
        w2e=W2e.reshape(2, 128, HID + 2).astype(TB_NP),         # [2, 128, 66]
        b0r=np.tile(b0, (128, 1)).astype(np.float32),
        b1r=np.tile(b1, (128, 1)).astype(np.float32),
        b2r=np.tile(b2, (128, 1)).astype(np.float32),
        iota_row=iota_row, iota_col=iota_col, ones1=ones1, ident=ident,
    )

    in_maps = []
    for c in range(NCORES):
        # xTb[b] = x[nodes of (c,b)].T : [128 feats, 128 slots]
        xtb = np.zeros((NB, IN_C, BS), np.float32)
        for b in range(NB):
            nb = pp["xperm"][_rowbase(c, b) + np.arange(BS)]
            valid = nb >= 0
            if valid.any():
                xtb[b][:, valid] = x[nb[valid]].T
        m = dict(
            xtb=xtb.astype(TB_NP),
            idx16=pp["idx16"][c],
            dstc=pp["dstc"][c],
            dstr=pp["dstr"][c],
            maskc=pp["maskc"][c],
            **consts,
        )
        in_maps.append(m)
    return in_maps


# ---------------- numpy emulation of the device data path ----------------

def _emulate_layer(tables_in, pp, We, brep, heads, F_out, relu, el):
    """tables_in: full node-major feature mat [RTOT, F_in].
    Returns out [RTOT, F_out] node-major post-activation."""
    Fo = F_out
    # transform (all rows; pad rows produce garbage but are never gathered)
    tb = tables_in @ We  # [RTOT, Fo + 2*heads]
    table = np.zeros((RTOT, el), TB_NP)
    table[:, :Fo + 2 * heads] = tb.astype(TB_NP)
    ad_all = tb[:, Fo + heads:Fo + 2 * heads].astype(TB_NP)  # [RTOT, heads]

    out = np.zeros((RTOT, Fo), np.float32)
    for c in range(NCORES):
        for bl in range(NB):
            rbase = _rowbase(c, bl)
            agg = np.zeros((BS, Fo), np.float32)
            den = np.zeros((BS, heads), np.float32)
            for kind in range(2):
                base = 0 if kind == 0 else HI_BASE
                cbase = (bl * 2 + kind) * NKCOLS
                w = pp["idx16"][c][:16, cbase:cbase + NKCOLS]
                rel = w.T.reshape(-1).astype(np.int64)  # unwrap
                dl = pp["dstr"][c][bl * 2 + kind].astype(np.int64)  # -1 pads
                valid = (dl >= 0) & (rel >= 0)
                rows = np.clip(rel + base, 0, RTOT - 1)
                g = np.asarray(table[rows], np.float32)  # [KE, el]
                a_s = g[:, Fo:Fo + heads]
                a_d = np.where(valid[:, None],
                               np.asarray(ad_all, np.float32)[
                                   rbase + np.clip(dl, 0, BS - 1)], 0.0)
                z = a_s + a_d
                s = np.exp(np.maximum(z, 0.2 * z)).astype(TB_NP)
                s = np.asarray(s, np.float32)
                hsc = (g[:, :Fo].reshape(KE, heads, HID)
                       * s[:, :, None]).astype(TB_NP).astype(np.float32)
                hsc = hsc.reshape(KE, Fo)
                np.add.at(agg, dl[valid], hsc[valid])
                np.add.at(den, dl[valid], s[valid])
            o = agg.reshape(BS, heads, HID) / (den + 1e-16)[:, :, None]
            o = o.reshape(BS, Fo) + brep[0]
            if relu:
                o = np.maximum(o, 0.0)
            out[rbase:rbase + BS] = o
    return out


def emulate(inputs, pp=None):
    """Full numpy emulation; returns [1, OUT_C]."""
    if pp is None:
        pp = preprocess(np.asarray(inputs["edge_index"]))
    x = np.asarray(inputs["x"], np.float32)
    W0e, W1e, W2e = host_weights(inputs)
    h = np.zeros((RTOT, IN_C), np.float32)
    valid = pp["xperm"] >= 0
    h[valid] = x[pp["xperm"][valid]]
    h = h.astype(TB_NP).astype(np.float32)

    b0r = np.tile(np.asarray(inputs["b0"], np.float32), (1, 1))
    b1r = np.tile(np.asarray(inputs["b1"], np.float32), (1, 1))
    b2r = np.tile(np.asarray(inputs["b2"], np.float32), (1, 1))

    h0 = _emulate_layer(h, pp, W0e.astype(TB_NP).astype(np.float32),
                        b0r, HEADS, F1, True, EL01)
    h1 = _emulate_layer(h0, pp, W1e.astype(TB_NP).astype(np.float32),
                        b1r, HEADS, F1, True, EL01)
    h2 = _emulate_layer(h1, pp, W2e.astype(TB_NP).astype(np.float32),
                        b2r, 1, HID, False, EL2)

    g = h2[valid].sum(axis=0, keepdims=True) / N
    return (g @ np.asarray(inputs["hw"], np.float32)
            + np.asarray(inputs["hb"], np.float32)).astype(np.float32)


# ---------------- device kernel ----------------

_BUILT = None


def build_kernel():
    import concourse.bacc as bacc
    import concourse.mybir as mybir
    import concourse.tile as tile
    from concourse import library_config

    f32 = mybir.dt.float32
    bf16 = mybir.dt.bfloat16
    i16 = mybir.dt.int16
    Alu = mybir.AluOpType
    Act = mybir.ActivationFunctionType

    nc = bacc.Bacc("TRN2", target_bir_lowering=False, debug=False,
                   num_devices=NCORES, num_swdge_queues=4)

    # ---- I/O ----
    xtb_d = nc.dram_tensor("xtb", [NB, IN_C, BS], bf16, kind="ExternalInput")
    idx16_d = nc.dram_tensor("idx16", [128, NB * 2 * NKCOLS], i16,
                             kind="ExternalInput")
    dstc_d = nc.dram_tensor("dstc", [128, NB * 2 * TL], bf16,
                            kind="ExternalInput")
    dstr_d = nc.dram_tensor("dstr", [128, KE], bf16, kind="ExternalInput")
    maskc_d = nc.dram_tensor("maskc", [128, NB], bf16, kind="ExternalInput")
    w0e_d = nc.dram_tensor("w0e", [IN_C, F1 + 2 * HEADS], bf16,
                           kind="ExternalInput")
    w1e_d = nc.dram_tensor("w1e", [2, 128, F1 + 2 * HEADS], bf16,
                           kind="ExternalInput")
    w2e_d = nc.dram_tensor("w2e", [2, 128, HID + 2], bf16,
                           kind="ExternalInput")
    b0r_d = nc.dram_tensor("b0r", [128, F1], f32, kind="ExternalInput")
    b1r_d = nc.dram_tensor("b1r", [128, F1], f32, kind="ExternalInput")
    b2r_d = nc.dram_tensor("b2r", [128, HID], f32, kind="ExternalInput")
    iota_row_d = nc.dram_tensor("iota_row", [128, 128], bf16,
                                kind="ExternalInput")
    iota_col_d = nc.dram_tensor("iota_col", [128, 1], bf16,
                                kind="ExternalInput")
    ones1_d = nc.dram_tensor("ones1", [1, 128], bf16, kind="ExternalInput")
    ident_d = nc.dram_tensor("ident", [128, 128], bf16, kind="ExternalInput")
    out_d = nc.dram_tensor("out_part", [1, OUT_C], f32, kind="ExternalOutput")

    # internal DRAM: shards + two ping-pong EL01 tables + the EL2 table
    shard0 = nc.dram_tensor("shard0", [NPC, EL01], bf16)
    shard1 = nc.dram_tensor("shard1", [NPC, EL01], bf16)
    shard2 = nc.dram_tensor("shard2", [NPC, EL2], bf16)
    tableA = nc.dram_tensor("tableA", [RTOT, EL01], bf16)
    tableB = nc.dram_tensor("tableB", [RTOT, EL01], bf16)
    table2 = nc.dram_tensor("table2", [RTOT, EL2], bf16)

    SHARDS = [shard0, shard1, shard2]
    TABLES = [tableA, tableB, table2]
    ELS = [EL01, EL01, EL2]
    HEADS_L = [HEADS, HEADS, 1]
    FO_L = [F1, F1, HID]

    rg = [list(range(NCORES))]

    qctr = [0]

    def next_q():
        q = qctr[0] & 3
        qctr[0] += 1
        return q

    with tile.TileContext(nc) as tc:
        with (
            tc.tile_pool(name="const", bufs=1) as cpool,
            tc.tile_pool(name="work", bufs=3) as wpool,
            tc.tile_pool(name="gather", bufs=3) as gpool,
            tc.tile_pool(name="small", bufs=4) as spool,
            tc.tile_pool(name="psum", bufs=2, space="PSUM") as ppool,
            tc.tile_pool(name="psum1", bufs=1, space="PSUM") as ppool1,
        ):
            # ---- load constants ----
            def load_const(tag, dram, shape, dtype=bf16, view=None):
                t = cpool.tile(shape, dtype, tag=tag)
                nc.sync.dma_start(out=t[:], in_=view if view is not None
                                  else dram[:])
                return t

            w0e_s = load_const("w0e", w0e_d, [IN_C, F1 + 2 * HEADS])
            w1e_s = load_const("w1e", w1e_d, [128, 2, F1 + 2 * HEADS],
                               view=w1e_d[:].rearrange("c p j -> p c j"))
            w2e_s = load_const("w2e", w2e_d, [128, 2, HID + 2],
                               view=w2e_d[:].rearrange("c p j -> p c j"))
            b0r_s = load_const("b0r", b0r_d, [128, F1], f32)
            b1r_s = load_const("b1r", b1r_d, [128, F1], f32)
            b2r_s = load_const("b2r", b2r_d, [128, HID], f32)
            iota_row_s = load_const("iota_row", iota_row_d, [128, 128])
            iota_col_s = load_const("iota_col", iota_col_d, [128, 1])
            ones1_s = load_const("ones1", ones1_d, [1, 128])
            ident_s = load_const("ident", ident_d, [128, 128])
            idx16_s = load_const("idx16", idx16_d,
                                 [128, NB * 2 * NKCOLS], i16)
            dstc_s = load_const("dstc", dstc_d, [128, NB * 2 * TL])
            maskc_s = load_const("maskc", maskc_d, [128, NB])
            WE = [w0e_s, w1e_s, w2e_s]
            BR = [b0r_s, b1r_s, b2r_s]

            nc.gpsimd.load_library(library_config.mlp)

            # ad_all per layer [128 slots, NB*heads] bf16
            ad_alls = [cpool.tile([128, NB * HEADS_L[l]], bf16,
                                  tag=f"ad_all{l}", name=f"ad_all{l}")
                       for l in range(3)]

            # init gather tiles so stripped pad rows never read NaN garbage
            for i in range(3):
                gz = gpool.tile([128, TL2, EL01], bf16, tag="g")
                nc.vector.memset(gz[:], 0.0)

            def transform_block(layer, b, src):
                """src: L0 -> xtb tile [128, 128]; else hT loc [128, 2, 128].
                Writes shard rows + ad_all column."""
                heads = HEADS_L[layer]
                Fo = FO_L[layer]
                ncols = Fo + 2 * heads
                el = ELS[layer]
                ps = ppool.tile([128, 512], f32, tag="tf", space="PSUM")
                if layer == 0:
                    nc.tensor.matmul(out=ps[:, :ncols], lhsT=src[:],
                                     rhs=w0e_s[:], start=True, stop=True)
                else:
                    we = WE[layer]
                    for k2 in range(2):
                        nc.tensor.matmul(
                            out=ps[:, :ncols], lhsT=src[:, k2, :],
                            rhs=we[:, k2, :],
                            start=(k2 == 0), stop=(k2 == 1))
                tb = wpool.tile([128, el], bf16, tag="tb")
                nc.vector.tensor_copy(out=tb[:, :ncols], in_=ps[:, :ncols])
                nc.vector.tensor_copy(
                    out=ad_alls[layer][:, b * heads:(b + 1) * heads],
                    in_=ps[:, Fo + heads:Fo + 2 * heads])
                nc.sync.dma_start(out=SHARDS[layer][b * BS:(b + 1) * BS, :],
                                  in_=tb[:])

            def allgather_chunk(layer, lo_b, hi_b):
                shard = SHARDS[layer]
                table = TABLES[layer]
                in_ap = shard[lo_b * BS:hi_b * BS, :]
                base = 8 * BS * lo_b   # chunk-major table: contiguous slab
                out_ap = table[base:base + 8 * (hi_b - lo_b) * BS, :]
                nc.gpsimd.collective_compute(
                    "AllGather", mybir.AluOpType.bypass,
                    replica_groups=rg, ins=[in_ap.opt()],
                    outs=[out_ap.opt()])

            def aggregate(layer):
                """Aggregation for `layer`; interleaves transform(layer+1)
                and its chunked allgather."""
                heads = HEADS_L[layer]
                Fo = FO_L[layer]
                el = ELS[layer]
                table = TABLES[layer]
                brep = BR[layer]
                views = [table[0:LO_LIM, :], table[HI_BASE:HI_BASE + 32768, :]]
                nxt = layer + 1
                chunk_i = [0]
                if layer == 2:
                    psum_sum = ppool1.tile([1, OUT_C], f32, tag="sum",
                                           space="PSUM")
                for b in range(NB):
                    g = gpool.tile([128, TL2, el], bf16, tag="g")
                    for kind in range(2):
                        bk = b * 2 + kind
                        nc.gpsimd.dma_gather(
                            g[:, kind * TL:(kind + 1) * TL, :], views[kind],
                            idx16_s[:, bk * NKCOLS:(bk + 1) * NKCOLS],
                            KE, KE, el, single_packet=False,
                            queue_num=next_q())
                    # one-hot M [128e, TL2*128d] over both kinds
                    M = wpool.tile([128, TL2 * 128], bf16, tag="M")
                    tcol = b * 2 * TL
                    nc.vector.tensor_tensor(
                        out=M[:].rearrange("p (t d) -> p t d", t=TL2),
                        in0=dstc_s[:, tcol:tcol + TL2].unsqueeze(-1)
                            .broadcast_to([128, TL2, 128]),
                        in1=iota_row_s[:].unsqueeze(1)
                            .broadcast_to([128, TL2, 128]),
                        op=Alu.is_equal)
                    # M_T [128d, TL2*128e] via replicated-row outer product
                    MT = wpool.tile([128, TL2 * 128], bf16, tag="MT")
                    dr = spool.tile([1, TL2 * 128], bf16, tag="dr")
                    nc.sync.dma_start(
                        out=dr[:],
                        in_=dstr_d[b * 2:b * 2 + 2, :]
                            .rearrange("p k -> (p k)").unsqueeze(0))
                    for o in range(0, TL2 * 128, 512):
                        wdt = min(512, TL2 * 128 - o)
                        pr = ppool1.tile([128, 512], f32, tag="rep",
                                         space="PSUM")
                        nc.tensor.matmul(out=pr[:, :wdt],
                                         lhsT=ones1_s[:],
                                         rhs=dr[:, o:o + wdt],
                                         start=True, stop=True)
                        nc.vector.tensor_tensor(
                            out=MT[:, o:o + wdt], in0=pr[:, :wdt],
                            in1=iota_col_s[:].broadcast_to([128, wdt]),
                            op=Alu.is_equal)
                    # ad per edge via M_T @ ad_block
                    pad_ = ppool1.tile([128, TL2 * heads], f32, tag="adp",
                                       space="PSUM")
                    for t in range(TL2):
                        nc.tensor.matmul(
                            out=pad_[:, t * heads:(t + 1) * heads],
                            lhsT=MT[:, t * 128:(t + 1) * 128],
                            rhs=ad_alls[layer][:, b * heads:(b + 1) * heads],
                            start=True, stop=True)
                    # z = as + ad ; s = exp(max(z, 0.2 z))
                    z = spool.tile([128, TL2 * heads], f32, tag="z")
                    nc.vector.tensor_tensor(
                        out=z[:].rearrange("p (t h) -> p t h", t=TL2),
                        in0=g[:, :, Fo:Fo + heads],
                        in1=pad_[:].rearrange("p (t h) -> p t h", t=TL2),
                        op=Alu.add)
                    z2 = spool.tile([128, TL2 * heads], f32, tag="z2")
                    nc.vector.tensor_scalar(out=z2[:], in0=z[:],
                                            scalar1=0.2, scalar2=None,
                                            op0=Alu.mult)
                    zm = spool.tile([128, TL2 * heads], f32, tag="zm")
                    nc.vector.tensor_tensor(out=zm[:], in0=z[:],
                                            in1=z2[:], op=Alu.max)
                    # tmp = [g * s | s]: s lands in the last `heads` columns
                    tmp = wpool.tile([128, TL2, Fo + heads], bf16, tag="tmp")
                    nc.scalar.activation(
                        tmp[:, :, Fo:Fo + heads],
                        zm[:].rearrange("p (t h) -> p t h", t=TL2), Act.Exp)
                    for hh in range(heads):
                        nc.vector.tensor_tensor(
                            out=tmp[:, :, hh * HID:(hh + 1) * HID],
                            in0=g[:, :, hh * HID:(hh + 1) * HID],
                            in1=tmp[:, :, Fo + hh:Fo + hh + 1]
                                .broadcast_to([128, TL2, HID]),
                            op=Alu.mult)
                    # accumulate features + denominator in one PSUM group
                    pagg = ppool.tile([128, Fo + heads], f32, tag="agg",
                                      space="PSUM")
                    for t in range(TL2):
                        nc.tensor.matmul(
                            out=pagg[:],
                            lhsT=M[:, t * 128:(t + 1) * 128],
                            rhs=tmp[:, t, :],
                            start=(t == 0), stop=(t == TL2 - 1))
                    # epilogue
                    den = spool.tile([128, heads], f32, tag="den")
                    nc.vector.tensor_scalar(out=den[:],
                                            in0=pagg[:, Fo:Fo + heads],
                                            scalar1=1e-16, scalar2=None,
                                            op0=Alu.add)
                    rec = spool.tile([128, heads], f32, tag="rec")
                    nc.vector.reciprocal(out=rec[:], in_=den[:])
                    o1 = wpool.tile([128, Fo], f32, tag="o1")
                    nc.vector.tensor_tensor(
                        out=o1[:].rearrange("p (h f) -> p h f", h=heads),
                        in0=pagg[:, :Fo].rearrange("p (h f) -> p h f",
                                                   h=heads),
                        in1=rec[:].unsqueeze(-1)
                            .broadcast_to([128, heads, HID]),
                        op=Alu.mult)
                    o2 = wpool.tile([128, Fo], bf16, tag="o2")
                    nc.vector.tensor_tensor(out=o2[:], in0=o1[:],
                                            in1=brep[:, :Fo], op=Alu.add)
                    if layer == 2:
                        nc.tensor.matmul(out=psum_sum[:],
                                         lhsT=maskc_s[:, b:b + 1],
                                         rhs=o2[:], start=(b == 0),
                                         stop=(b == NB - 1))
                    else:
                        o3 = wpool.tile([128, Fo], bf16, tag="o3")
                        nc.scalar.activation(o3[:], o2[:], Act.Relu)
                        hloc = wpool.tile([128, 2, 128], bf16, tag="hloc")
                        for k2 in range(2):
                            pt = ppool1.tile([128, 128], bf16, tag="tp",
                                             space="PSUM")
                            nc.tensor.transpose(
                                pt[:], o3[:, k2 * 128:(k2 + 1) * 128],
                                ident_s[:])
                            nc.vector.tensor_copy(out=hloc[:, k2, :],
                                                  in_=pt[:])
                        transform_block(nxt, b, hloc)
                        if b + 1 == CHUNK_BOUNDS[chunk_i[0] + 1]:
                            allgather_chunk(nxt, CHUNK_BOUNDS[chunk_i[0]],
                                            CHUNK_BOUNDS[chunk_i[0] + 1])
                            chunk_i[0] += 1
                if layer == 2:
                    osb = spool.tile([1, OUT_C], f32, tag="osb")
                    nc.vector.tensor_copy(out=osb[:], in_=psum_sum[:])
                    nc.sync.dma_start(out=out_d[:], in_=osb[:])

            # ---- layer 0 setup: transform own shard + chunked allgather ----
            ci = 0
            for b in range(NB):
                xb = wpool.tile([IN_C, BS], bf16, tag="xtb")
                nc.sync.dma_start(out=xb[:], in_=xtb_d[b])
                transform_block(0, b, xb)
                if b + 1 == CHUNK_BOUNDS[ci + 1]:
                    allgather_chunk(0, CHUNK_BOUNDS[ci], CHUNK_BOUNDS[ci + 1])
                    ci += 1

            aggregate(0)   # interleaves transform(1) + allgather chunks
            aggregate(1)   # interleaves transform(2) + allgather chunks
            aggregate(2)

    nc.compile()
    return nc


def _get_built():
    global _BUILT
    if _BUILT is None:
        _BUILT = build_kernel()
    return _BUILT


def kernel(**inputs) -> np.ndarray:
    from concourse.bass_utils import run_bass_kernel_spmd

    pp = preprocess(np.asarray(inputs["edge_index"]))
    in_maps = build_core_inputs(inputs, pp)
    nc = _get_built()
    res = run_bass_kernel_spmd(nc, in_maps, core_ids=list(range(NCORES)))
    parts = np.stack([r["out_part"][0] for r in res.results])  # [8, 64]
    g = parts.sum(axis=0, keepdims=True) / N
    out = (g @ np.asarray(inputs["hw"], np.float32)
           + np.asarray(inputs["hb"], np.float32)).astype(np.float32)
    return out


# revision 16
# speedup vs baseline: 1.4446x; 1.4446x over previous
"""3-layer GAT on 8 trn2 NeuronCores.

Strategy (graph/data parallel per sharding hint):
  - Nodes are assigned to 8 cores x 49 blocks x 128 slots (degree-balanced
    LPT bin packing) -> permuted node order; "table row" = block*128 + slot.
  - Per layer: each core transforms its own node shard with
    rhs = [W | W@as | W@ad] (alpha terms folded into the matmul), writes a
    bf16 table shard [6272, EL], chunked AllGather -> full table on every
    core (chunks overlap with the previous layer's aggregation).
  - Aggregation: per dst-block of 128 nodes, edges (dst-sorted) are packed
    into 128-edge tiles; a dma_gather fetches table rows for the tile's
    sources; a one-hot "scatter matrix" matmul accumulates the s_e-weighted
    features AND the softmax denominator (s appended as extra rhs columns)
    into PSUM.  (Softmax max-shift is skipped: logits are O(1) so exp is
    safe, and the result is mathematically identical.)
  - int16 gather indices: table split into lo rows [0,32768) and hi rows
    [17408,50176); per-block edges are balanced between the (overlapping)
    windows so each side fits 9 tiles of 128.  Pad slots get index -1 so
    the Q7 descriptor generator strips them (trailing negatives).
  - dma_gather calls rotate across 4 SWDGE queues so descriptor generation
    parallelizes over the Q7 core pairs.
  - Transform of layer L+1 for block b runs right after block b's layer-L
    epilogue (per-block hT temp, no big hT buffer); shard chunks AllGather
    while later blocks still aggregate.
  - Layer 2 output is column-summed per core (masked for pad slots); the
    final mean + linear head run on host.
"""

import os
import numpy as np
import ml_dtypes

# ---------------- problem constants (must match reference) ----------------
N = 50000
E = 800000
IN_C = 128
HID = 64
HEADS = 4
OUT_C = 64
F1 = HEADS * HID  # 256

# ---------------- sharding geometry ----------------
NCORES = 8
NB = 49           # dst blocks per core
BS = 128          # dst slots per block
NPC = NB * BS     # 6272 nodes per core
RTOT = NCORES * NPC  # 50176 table rows
TL = 9            # tiles per kind (lo/hi)
TL2 = 2 * TL      # tiles per block (both kinds)
KE = TL * 128     # 1152 edge slots per (block, kind)
LO_LIM = 32768    # lo window rows [0, LO_LIM)
HI_BASE = 17408   # hi window rows [HI_BASE, HI_BASE+32768)
NKCOLS = KE // 16  # 72 idx columns per (block, kind)

TB_NP = ml_dtypes.bfloat16
EL01 = 384     # table elems/row layer0/1 (256 h + 4 as + 4 ad + pad), bf16
EL2 = 128      # table elems/row layer2 (64 h + 1 as + 1 ad + pad), bf16

NCHUNK = 4     # allgather chunks per layer
CHUNK_BOUNDS = [0, 13, 26, 38, 49]


def _chunk_of(b):
    for k in range(NCHUNK):
        if b < CHUNK_BOUNDS[k + 1]:
            return k
    raise ValueError(b)


def _rowbase(c, b):
    """Table row of (core c, block b, slot 0) in chunk-major table layout
    [chunk, core, blocks_in_chunk, slot] — keeps each chunk's AllGather
    output contiguous."""
    k = _chunk_of(b)
    base = 8 * BS * CHUNK_BOUNDS[k]
    ck = (CHUNK_BOUNDS[k + 1] - CHUNK_BOUNDS[k]) * BS
    return base + c * ck + (b - CHUNK_BOUNDS[k]) * BS


# ---------------- host preprocessing ----------------

def preprocess(edge_index):
    """Node->(core,block,slot) assignment and per-core edge tile arrays.

    Returns dict with:
      row:   [N] table row of each node
      xperm: [RTOT] node id occupying each table row (-1 for pad slots)
      idx16: [NCORES,128,NB*2*NKCOLS] int16 wrapped gather indices (-1 pad)
      dstc:  [NCORES,128,NB*2*TL] bf16 dst_local per edge slot (col layout,
             -1 pad)
      dstr:  [NCORES,128,KE] bf16 dst_local (row layout; partition=blk*2+kind)
      maskc: [NCORES,128,NB] bf16 1.0 for real-node slots
    """
    import heapq

    src = np.concatenate([np.asarray(edge_index[0]), np.arange(N, dtype=np.int64)])
    dst = np.concatenate([np.asarray(edge_index[1]), np.arange(N, dtype=np.int64)])
    deg = np.bincount(dst, minlength=N)

    nblocks = NCORES * NB
    order = np.argsort(-deg, kind="stable")
    heap = [(0, b) for b in range(nblocks)]
    heapq.heapify(heap)
    slots_used = np.zeros(nblocks, np.int64)
    node_block = np.empty(N, np.int64)
    node_slot = np.empty(N, np.int64)
    for n in order:
        while True:
            load, b = heapq.heappop(heap)
            if slots_used[b] < BS:
                break
        node_block[n] = b
        node_slot[n] = slots_used[b]
        slots_used[b] += 1
        heapq.heappush(heap, (load + int(deg[n]), b))
        # blocks that were full stay out of the heap

    # table row per node: chunk-major layout (see _rowbase)
    rowbase_cb = np.empty(nblocks, np.int64)
    for b in range(nblocks):
        c, bl = divmod(b, NB)
        rowbase_cb[b] = _rowbase(c, bl)
    row = rowbase_cb[node_block] + node_slot

    xperm = np.full(RTOT, -1, np.int64)
    xperm[row] = np.arange(N)

    erow = row[src]          # gather row per edge
    eblk = node_block[dst]   # destination block per edge
    eslot = node_slot[dst]   # dst_local per edge

    idx16 = np.zeros((NCORES, 128, NB * 2 * NKCOLS), np.int16)
    dstc = np.full((NCORES, 128, NB * 2 * TL), -1.0, TB_NP)
    dstr = np.zeros((NCORES, 128, KE), TB_NP)
    maskc = np.zeros((NCORES, 128, NB), TB_NP)

    order_e = np.argsort(eblk, kind="stable")
    bounds = np.searchsorted(eblk[order_e], np.arange(nblocks + 1))

    for b in range(nblocks):
        c, bl = divmod(b, NB)
        es = order_e[bounds[b]:bounds[b + 1]]
        r_ = erow[es]
        dl = eslot[es]
        lo_f = r_ < HI_BASE
        hi_f = r_ >= LO_LIM
        flex = ~lo_f & ~hi_f
        n_lo = int(lo_f.sum())
        n_hi = int(hi_f.sum())
        n_fx = int(flex.sum())
        tot = n_lo + n_hi + n_fx
        assert tot <= 2 * KE, f"block {b} has {tot} edges > {2*KE}"
        # send flex edges to lo until lo reaches ceil(tot/2) (capped at KE)
        add_lo = min(n_fx, max(0, min(KE, (tot + 1) // 2) - n_lo))
        if n_hi + (n_fx - add_lo) > KE:
            add_lo = n_fx - (KE - n_hi)
        assert 0 <= add_lo <= n_fx
        fx_idx = np.nonzero(flex)[0]
        sel_lo = np.zeros(len(es), bool)
        sel_lo[lo_f] = True
        sel_lo[fx_idx[:add_lo]] = True
        sel_hi = ~sel_lo
        assert sel_lo.sum() <= KE and sel_hi.sum() <= KE, (
            b, sel_lo.sum(), sel_hi.sum())

        for kind, sel, base in ((0, sel_lo, 0), (1, sel_hi, HI_BASE)):
            rr = r_[sel]
            dd = dl[sel]
            o = np.argsort(rr, kind="stable")  # DMA locality
            rr = rr[o]
            dd = dd[o]
            k = len(rr)
            rel = np.full(KE, -1, np.int64)   # -1 pads: Q7 strips trailing
            rel[:k] = rr - base
            dloc = np.full(KE, -1.0, np.float32)
            dloc[:k] = dd.astype(np.float32)
            assert k == 0 or (rel[:k].min() >= 0 and rel[:k].max() < 32768)
            # wrapped idx: index i -> [i % 16, i // 16]
            w = rel.reshape(NKCOLS, 16).T.astype(np.int16)  # [16, NKCOLS]
            cbase = (bl * 2 + kind) * NKCOLS
            idx16[c, :, cbase:cbase + NKCOLS] = np.tile(w, (8, 1))
            # col layout: col bl*2*TL + kind*TL + t, partition p = edge t*128+p
            tcol = bl * 2 * TL + kind * TL
            dstc[c, :, tcol:tcol + TL] = dloc.reshape(TL, 128).T.astype(TB_NP)
            # row layout: partition bl*2+kind
            dstr[c, bl * 2 + kind, :] = dloc.astype(TB_NP)

        # mask of real slots
        used = slots_used[b]
        maskc[c, :used, bl] = 1.0

    return dict(row=row, xperm=xperm, idx16=idx16, dstc=dstc, dstr=dstr,
                maskc=maskc, deg=deg, node_block=node_block,
                node_slot=node_slot)


def host_weights(inputs):
    """Extended weight matrices with folded attention vectors."""
    def ext(W, a_s, a_d, heads):
        # Was[k, h] = sum_c W[k, h*HID+c] * a_s[h, c]
        Wh = W.reshape(W.shape[0], heads, HID)
        Was = np.einsum("khc,hc->kh", Wh, a_s)
        Wad = np.einsum("khc,hc->kh", Wh, a_d)
        return np.concatenate([W, Was, Wad], axis=1).astype(np.float32)

    W0e = ext(np.asarray(inputs["W0"], np.float32),
              np.asarray(inputs["a0s"], np.float32),
              np.asarray(inputs["a0d"], np.float32), HEADS)      # [128, 264]
    W1e = ext(np.asarray(inputs["W1"], np.float32),
              np.asarray(inputs["a1s"], np.float32),
              np.asarray(inputs["a1d"], np.float32), HEADS)      # [256, 264]
    W2e = ext(np.asarray(inputs["W2"], np.float32),
              np.asarray(inputs["a2s"], np.float32),
              np.asarray(inputs["a2d"], np.float32), 1)          # [256, 66]
    return W0e, W1e, W2e


def build_core_inputs(inputs, pp):
    """Per-core in_maps for run_bass_kernel_spmd."""
    x = np.asarray(inputs["x"], np.float32)
    W0e, W1e, W2e = host_weights(inputs)
    b0 = np.asarray(inputs["b0"], np.float32)
    b1 = np.asarray(inputs["b1"], np.float32)
    b2 = np.asarray(inputs["b2"], np.float32)

    iota_row = np.tile(np.arange(128, dtype=np.float32), (128, 1)).astype(TB_NP)
    iota_col = np.arange(128, dtype=np.float32).reshape(128, 1).astype(TB_NP)
    ones1 = np.ones((1, 128), TB_NP)
    ident = np.eye(128, dtype=np.float32).astype(TB_NP)

    consts = dict(
        w0e=W0e.astype(TB_NP),                                  # [128, 264]
        w1e=W1e.reshape(2, 128, F1 + 2 * HEADS).astype(TB_NP),  # [2, 128, 264]
        w2e=W2e.reshape(2, 128, HID + 2).astype(TB_NP),         # [2, 128, 66]
        b0r=np.tile(b0, (128, 1)).astype(np.float32),
        b1r=np.tile(b1, (128, 1)).astype(np.float32),
        b2r=np.tile(b2, (128, 1)).astype(np.float32),
        iota_row=iota_row, iota_col=iota_col, ones1=ones1, ident=ident,
    )

    in_maps = []
    for c in range(NCORES):
        # xTb[b] = x[nodes of (c,b)].T : [128 feats, 128 slots]
        xtb = np.zeros((NB, IN_C, BS), np.float32)
        for b in range(NB):
            nb = pp["xperm"][_rowbase(c, b) + np.arange(BS)]
            valid = nb >= 0
            if valid.any():
                xtb[b][:, valid] = x[nb[valid]].T
        m = dict(
            xtb=xtb.astype(TB_NP),
            idx16=pp["idx16"][c],
            dstc=pp["dstc"][c],
            dstr=pp["dstr"][c],
            maskc=pp["maskc"][c],
            **consts,
        )
        in_maps.append(m)
    return in_maps


# ---------------- numpy emulation of the device data path ----------------

def _emulate_layer(tables_in, pp, We, brep, heads, F_out, relu, el):
    """tables_in: full node-major feature mat [RTOT, F_in].
    Returns out [RTOT, F_out] node-major post-activation."""
    Fo = F_out
    # transform (all rows; pad rows produce garbage but are never gathered)
    tb = tables_in @ We  # [RTOT, Fo + 2*heads]
    table = np.zeros((RTOT, el), TB_NP)
    table[:, :Fo + 2 * heads] = tb.astype(TB_NP)
    ad_all = tb[:, Fo + heads:Fo + 2 * heads].astype(TB_NP)  # [RTOT, heads]

    out = np.zeros((RTOT, Fo), np.float32)
    for c in range(NCORES):
        for bl in range(NB):
            rbase = _rowbase(c, bl)
            agg = np.zeros((BS, Fo), np.float32)
            den = np.zeros((BS, heads), np.float32)
            for kind in range(2):
                base = 0 if kind == 0 else HI_BASE
                cbase = (bl * 2 + kind) * NKCOLS
                w = pp["idx16"][c][:16, cbase:cbase + NKCOLS]
                rel = w.T.reshape(-1).astype(np.int64)  # unwrap
                dl = pp["dstr"][c][bl * 2 + kind].astype(np.int64)  # -1 pads
                valid = (dl >= 0) & (rel >= 0)
                rows = np.clip(rel + base, 0, RTOT - 1)
                g = np.asarray(table[rows], np.float32)  # [KE, el]
                a_s = g[:, Fo:Fo + heads]
                a_d = np.where(valid[:, None],
                               np.asarray(ad_all, np.float32)[
                                   rbase + np.clip(dl, 0, BS - 1)], 0.0)
                z = a_s + a_d
                s = np.exp(np.maximum(z, 0.2 * z)).astype(TB_NP)
                s = np.asarray(s, np.float32)
                hsc = (g[:, :Fo].reshape(KE, heads, HID)
                       * s[:, :, None]).astype(TB_NP).astype(np.float32)
                hsc = hsc.reshape(KE, Fo)
                np.add.at(agg, dl[valid], hsc[valid])
                np.add.at(den, dl[valid], s[valid])
            o = agg.reshape(BS, heads, HID) / (den + 1e-16)[:, :, None]
            o = o.reshape(BS, Fo) + brep[0]
            if relu:
                o = np.maximum(o, 0.0)
            out[rbase:rbase + BS] = o
    return out


def emulate(inputs, pp=None):
    """Full numpy emulation; returns [1, OUT_C]."""
    if pp is None:
        pp = preprocess(np.asarray(inputs["edge_index"]))
    x = np.asarray(inputs["x"], np.float32)
    W0e, W1e, W2e = host_weights(inputs)
    h = np.zeros((RTOT, IN_C), np.float32)
    valid = pp["xperm"] >= 0
    h[valid] = x[pp["xperm"][valid]]
    h = h.astype(TB_NP).astype(np.float32)

    b0r = np.tile(np.asarray(inputs["b0"], np.float32), (1, 1))
    b1r = np.tile(np.asarray(inputs["b1"], np.float32), (1, 1))
    b2r = np.tile(np.asarray(inputs["b2"], np.float32), (1, 1))

    h0 = _emulate_layer(h, pp, W0e.astype(TB_NP).astype(np.float32),
                        b0r, HEADS, F1, True, EL01)
    h1 = _emulate_layer(h0, pp, W1e.astype(TB_NP).astype(np.float32),
                        b1r, HEADS, F1, True, EL01)
    h2 = _emulate_layer(h1, pp, W2e.astype(TB_NP).astype(np.float32),
                        b2r, 1, HID, False, EL2)

    g = h2[valid].sum(axis=0, keepdims=True) / N
    return (g @ np.asarray(inputs["hw"], np.float32)
            + np.asarray(inputs["hb"], np.float32)).astype(np.float32)


# ---------------- device kernel ----------------

_BUILT = None


def build_kernel():
    import concourse.bacc as bacc
    import concourse.mybir as mybir
    import concourse.tile as tile
    from concourse import library_config

    f32 = mybir.dt.float32
    bf16 = mybir.dt.bfloat16
    i16 = mybir.dt.int16
    Alu = mybir.AluOpType
    Act = mybir.ActivationFunctionType

    nc = bacc.Bacc("TRN2", target_bir_lowering=False, debug=False,
                   num_devices=NCORES, num_swdge_queues=4)

    # ---- I/O ----
    xtb_d = nc.dram_tensor("xtb", [NB, IN_C, BS], bf16, kind="ExternalInput")
    idx16_d = nc.dram_tensor("idx16", [128, NB * 2 * NKCOLS], i16,
                             kind="ExternalInput")
    dstc_d = nc.dram_tensor("dstc", [128, NB * 2 * TL], bf16,
                            kind="ExternalInput")
    dstr_d = nc.dram_tensor("dstr", [128, KE], bf16, kind="ExternalInput")
    maskc_d = nc.dram_tensor("maskc", [128, NB], bf16, kind="ExternalInput")
    w0e_d = nc.dram_tensor("w0e", [IN_C, F1 + 2 * HEADS], bf16,
                           kind="ExternalInput")
    w1e_d = nc.dram_tensor("w1e", [2, 128, F1 + 2 * HEADS], bf16,
                           kind="ExternalInput")
    w2e_d = nc.dram_tensor("w2e", [2, 128, HID + 2], bf16,
                           kind="ExternalInput")
    b0r_d = nc.dram_tensor("b0r", [128, F1], f32, kind="ExternalInput")
    b1r_d = nc.dram_tensor("b1r", [128, F1], f32, kind="ExternalInput")
    b2r_d = nc.dram_tensor("b2r", [128, HID], f32, kind="ExternalInput")
    iota_row_d = nc.dram_tensor("iota_row", [128, 128], bf16,
                                kind="ExternalInput")
    iota_col_d = nc.dram_tensor("iota_col", [128, 1], bf16,
                                kind="ExternalInput")
    ones1_d = nc.dram_tensor("ones1", [1, 128], bf16, kind="ExternalInput")
    ident_d = nc.dram_tensor("ident", [128, 128], bf16, kind="ExternalInput")
    out_d = nc.dram_tensor("out_part", [1, OUT_C], f32, kind="ExternalOutput")

    # internal DRAM: shards + two ping-pong EL01 tables + the EL2 table
    shard0 = nc.dram_tensor("shard0", [NPC, EL01], bf16)
    shard1 = nc.dram_tensor("shard1", [NPC, EL01], bf16)
    shard2 = nc.dram_tensor("shard2", [NPC, EL2], bf16)
    tableA = nc.dram_tensor("tableA", [RTOT, EL01], bf16)
    tableB = nc.dram_tensor("tableB", [RTOT, EL01], bf16)
    table2 = nc.dram_tensor("table2", [RTOT, EL2], bf16)

    SHARDS = [shard0, shard1, shard2]
    TABLES = [tableA, tableB, table2]
    ELS = [EL01, EL01, EL2]
    HEADS_L = [HEADS, HEADS, 1]
    FO_L = [F1, F1, HID]

    rg = [list(range(NCORES))]

    qctr = [0]

    def next_q():
        q = qctr[0] & 3
        qctr[0] += 1
        return q

    with tile.TileContext(nc) as tc:
        with (
            tc.tile_pool(name="const", bufs=1) as cpool,
            tc.tile_pool(name="work", bufs=3) as wpool,
            tc.tile_pool(name="gather", bufs=5) as gpool,
            tc.tile_pool(name="small", bufs=4) as spool,
            tc.tile_pool(name="psum", bufs=2, space="PSUM") as ppool,
            tc.tile_pool(name="psum1", bufs=1, space="PSUM") as ppool1,
        ):
            # ---- load constants ----
            def load_const(tag, dram, shape, dtype=bf16, view=None):
                t = cpool.tile(shape, dtype, tag=tag)
                nc.sync.dma_start(out=t[:], in_=view if view is not None
                                  else dram[:])
                return t

            w0e_s = load_const("w0e", w0e_d, [IN_C, F1 + 2 * HEADS])
            w1e_s = load_const("w1e", w1e_d, [128, 2, F1 + 2 * HEADS],
                               view=w1e_d[:].rearrange("c p j -> p c j"))
            w2e_s = load_const("w2e", w2e_d, [128, 2, HID + 2],
                               view=w2e_d[:].rearrange("c p j -> p c j"))
            b0r_s = load_const("b0r", b0r_d, [128, F1], f32)
            b1r_s = load_const("b1r", b1r_d, [128, F1], f32)
            b2r_s = load_const("b2r", b2r_d, [128, HID], f32)
            iota_row_s = load_const("iota_row", iota_row_d, [128, 128])
            iota_col_s = load_const("iota_col", iota_col_d, [128, 1])
            ones1_s = load_const("ones1", ones1_d, [1, 128])
            ident_s = load_const("ident", ident_d, [128, 128])
            idx16_s = load_const("idx16", idx16_d,
                                 [128, NB * 2 * NKCOLS], i16)
            dstc_s = load_const("dstc", dstc_d, [128, NB * 2 * TL])
            maskc_s = load_const("maskc", maskc_d, [128, NB])
            WE = [w0e_s, w1e_s, w2e_s]
            BR = [b0r_s, b1r_s, b2r_s]

            nc.gpsimd.load_library(library_config.mlp)

            # ad_all per layer [128 slots, NB*heads] bf16
            ad_alls = [cpool.tile([128, NB * HEADS_L[l]], bf16,
                                  tag=f"ad_all{l}", name=f"ad_all{l}")
                       for l in range(3)]

            # init gather tiles so stripped pad rows never read NaN garbage
            # (must cover every rotating buffer of the 'g' tag)
            for i in range(5):
                gz = gpool.tile([128, TL2, EL01], bf16, tag="g")
                nc.vector.memset(gz[:], 0.0)

            def transform_block(layer, b, src):
                """src: L0 -> xtb tile [128, 128]; else hT loc [128, 2, 128].
                Writes shard rows + ad_all column."""
                heads = HEADS_L[layer]
                Fo = FO_L[layer]
                ncols = Fo + 2 * heads
                el = ELS[layer]
                ps = ppool.tile([128, 512], f32, tag="tf", space="PSUM",
                                bufs=1)
                if layer == 0:
                    nc.tensor.matmul(out=ps[:, :ncols], lhsT=src[:],
                                     rhs=w0e_s[:], start=True, stop=True)
                else:
                    we = WE[layer]
                    for k2 in range(2):
                        nc.tensor.matmul(
                            out=ps[:, :ncols], lhsT=src[:, k2, :],
                            rhs=we[:, k2, :],
                            start=(k2 == 0), stop=(k2 == 1))
                tb = wpool.tile([128, el], bf16, tag="tb")
                nc.vector.tensor_copy(out=tb[:, :ncols], in_=ps[:, :ncols])
                nc.vector.tensor_copy(
                    out=ad_alls[layer][:, b * heads:(b + 1) * heads],
                    in_=ps[:, Fo + heads:Fo + 2 * heads])
                nc.sync.dma_start(out=SHARDS[layer][b * BS:(b + 1) * BS, :],
                                  in_=tb[:])

            def allgather_chunk(layer, lo_b, hi_b):
                shard = SHARDS[layer]
                table = TABLES[layer]
                in_ap = shard[lo_b * BS:hi_b * BS, :]
                base = 8 * BS * lo_b   # chunk-major table: contiguous slab
                out_ap = table[base:base + 8 * (hi_b - lo_b) * BS, :]
                nc.gpsimd.collective_compute(
                    "AllGather", mybir.AluOpType.bypass,
                    replica_groups=rg, ins=[in_ap.opt()],
                    outs=[out_ap.opt()])

            def aggregate(layer):
                """Aggregation for `layer`; interleaves transform(layer+1)
                and its chunked allgather."""
                heads = HEADS_L[layer]
                Fo = FO_L[layer]
                el = ELS[layer]
                table = TABLES[layer]
                brep = BR[layer]
                views = [table[0:LO_LIM, :], table[HI_BASE:HI_BASE + 32768, :]]
                nxt = layer + 1
                chunk_i = [0]
                if layer == 2:
                    psum_sum = ppool1.tile([1, OUT_C], f32, tag="sum",
                                           space="PSUM")

                def pre(b):
                    """Gather + per-edge weights for block b -> (M, tmp)."""
                    g = gpool.tile([128, TL2, el], bf16, tag="g",
                                   name=f"g{layer}_{b}")
                    for kind in range(2):
                        bk = b * 2 + kind
                        nc.gpsimd.dma_gather(
                            g[:, kind * TL:(kind + 1) * TL, :], views[kind],
                            idx16_s[:, bk * NKCOLS:(bk + 1) * NKCOLS],
                            KE, KE, el, single_packet=False,
                            queue_num=next_q())
                    # one-hot M [128e, TL2*128d] over both kinds
                    M = wpool.tile([128, TL2 * 128], bf16, tag="M",
                                   name=f"M{layer}_{b}")
                    tcol = b * 2 * TL
                    nc.vector.tensor_tensor(
                        out=M[:].rearrange("p (t d) -> p t d", t=TL2),
                        in0=dstc_s[:, tcol:tcol + TL2].unsqueeze(-1)
                            .broadcast_to([128, TL2, 128]),
                        in1=iota_row_s[:].unsqueeze(1)
                            .broadcast_to([128, TL2, 128]),
                        op=Alu.is_equal)
                    # M_T [128d, TL2*128e] via replicated-row outer product
                    MT = wpool.tile([128, TL2 * 128], bf16, tag="MT",
                                    name=f"MT{layer}_{b}")
                    dr = spool.tile([1, TL2 * 128], bf16, tag="dr",
                                    name=f"dr{layer}_{b}")
                    nc.sync.dma_start(
                        out=dr[:],
                        in_=dstr_d[b * 2:b * 2 + 2, :]
                            .rearrange("p k -> (p k)").unsqueeze(0))
                    for o in range(0, TL2 * 128, 512):
                        wdt = min(512, TL2 * 128 - o)
                        pr = ppool1.tile([128, 512], f32, tag="rep",
                                         space="PSUM", bufs=2,
                                         name=f"pr{layer}_{b}_{o}")
                        nc.tensor.matmul(out=pr[:, :wdt],
                                         lhsT=ones1_s[:],
                                         rhs=dr[:, o:o + wdt],
                                         start=True, stop=True)
                        nc.vector.tensor_tensor(
                            out=MT[:, o:o + wdt], in0=pr[:, :wdt],
                            in1=iota_col_s[:].broadcast_to([128, wdt]),
                            op=Alu.is_equal)
                    # ad per edge via M_T @ ad_block
                    pad_ = ppool1.tile([128, TL2 * heads], f32, tag="adp",
                                       space="PSUM", name=f"pad{layer}_{b}")
                    for t in range(TL2):
                        nc.tensor.matmul(
                            out=pad_[:, t * heads:(t + 1) * heads],
                            lhsT=MT[:, t * 128:(t + 1) * 128],
                            rhs=ad_alls[layer][:, b * heads:(b + 1) * heads],
                            start=True, stop=True)
                    # z = as + ad ; s = exp(leaky(z))
                    z = spool.tile([128, TL2 * heads], f32, tag="z",
                                   name=f"z{layer}_{b}")
                    nc.vector.tensor_tensor(
                        out=z[:].rearrange("p (t h) -> p t h", t=TL2),
                        in0=g[:, :, Fo:Fo + heads],
                        in1=pad_[:].rearrange("p (t h) -> p t h", t=TL2),
                        op=Alu.add)
                    zm = spool.tile([128, TL2 * heads], f32, tag="zm",
                                    name=f"zm{layer}_{b}")
                    nc.scalar.activation(zm[:], z[:], Act.Lrelu, alpha=0.2)
                    # tmp = [g * s | s]: s lands in the last `heads` columns
                    tmp = wpool.tile([128, TL2, Fo + heads], bf16, tag="tmp",
                                     name=f"tmp{layer}_{b}")
                    nc.scalar.activation(
                        tmp[:, :, Fo:Fo + heads],
                        zm[:].rearrange("p (t h) -> p t h", t=TL2), Act.Exp)
                    for hh in range(heads):
                        nc.vector.tensor_tensor(
                            out=tmp[:, :, hh * HID:(hh + 1) * HID],
                            in0=g[:, :, hh * HID:(hh + 1) * HID],
                            in1=tmp[:, :, Fo + hh:Fo + hh + 1]
                                .broadcast_to([128, TL2, HID]),
                            op=Alu.mult)
                    return M, tmp

                def post(b, M, tmp):
                    # accumulate features + denominator in one PSUM group
                    pagg = ppool.tile([128, Fo + heads], f32, tag="agg",
                                      space="PSUM", name=f"pagg{layer}_{b}")
                    for t in range(TL2):
                        nc.tensor.matmul(
                            out=pagg[:],
                            lhsT=M[:, t * 128:(t + 1) * 128],
                            rhs=tmp[:, t, :],
                            start=(t == 0), stop=(t == TL2 - 1))
                    den = spool.tile([128, heads], f32, tag="den",
                                     name=f"den{layer}_{b}")
                    nc.vector.tensor_scalar(out=den[:],
                                            in0=pagg[:, Fo:Fo + heads],
                                            scalar1=1e-16, scalar2=None,
                                            op0=Alu.add)
                    rec = spool.tile([128, heads], f32, tag="rec",
                                     name=f"rec{layer}_{b}")
                    nc.vector.reciprocal(out=rec[:], in_=den[:])
                    o1 = wpool.tile([128, Fo], f32, tag="o1",
                                    name=f"o1{layer}_{b}")
                    nc.vector.tensor_tensor(
                        out=o1[:].rearrange("p (h f) -> p h f", h=heads),
                        in0=pagg[:, :Fo].rearrange("p (h f) -> p h f",
                                                   h=heads),
                        in1=rec[:].unsqueeze(-1)
                            .broadcast_to([128, heads, HID]),
                        op=Alu.mult)
                    o2 = wpool.tile([128, Fo], bf16, tag="o2",
                                    name=f"o2{layer}_{b}")
                    nc.vector.tensor_tensor(out=o2[:], in0=o1[:],
                                            in1=brep[:, :Fo], op=Alu.add)
                    if layer == 2:
                        nc.tensor.matmul(out=psum_sum[:],
                                         lhsT=maskc_s[:, b:b + 1],
                                         rhs=o2[:], start=(b == 0),
                                         stop=(b == NB - 1))
                    else:
                        o3 = wpool.tile([128, Fo], bf16, tag="o3",
                                        name=f"o3{layer}_{b}")
                        nc.scalar.activation(o3[:], o2[:], Act.Relu)
                        hloc = wpool.tile([128, 2, 128], bf16, tag="hloc",
                                          name=f"hloc{layer}_{b}")
                        for k2 in range(2):
                            pt = ppool1.tile([128, 128], bf16, tag="tp",
                                             space="PSUM",
                                             name=f"pt{layer}_{b}_{k2}")
                            nc.tensor.transpose(
                                pt[:], o3[:, k2 * 128:(k2 + 1) * 128],
                                ident_s[:])
                            nc.vector.tensor_copy(out=hloc[:, k2, :],
                                                  in_=pt[:])
                        transform_block(nxt, b, hloc)
                        if b + 1 == CHUNK_BOUNDS[chunk_i[0] + 1]:
                            allgather_chunk(nxt, CHUNK_BOUNDS[chunk_i[0]],
                                            CHUNK_BOUNDS[chunk_i[0] + 1])
                            chunk_i[0] += 1

                # software-pipelined: PRE(b+1) is emitted before POST(b) so
                # the tensor engine never stalls on same-block DVE results
                carry = pre(0)
                for b in range(NB):
                    nxt_carry = pre(b + 1) if b + 1 < NB else None
                    post(b, *carry)
                    carry = nxt_carry
                if layer == 2:
                    osb = spool.tile([1, OUT_C], f32, tag="osb")
                    nc.vector.tensor_copy(out=osb[:], in_=psum_sum[:])
                    nc.sync.dma_start(out=out_d[:], in_=osb[:])

            # ---- layer 0 setup: transform own shard + chunked allgather ----
            ci = 0
            for b in range(NB):
                xb = wpool.tile([IN_C, BS], bf16, tag="xtb")
                nc.sync.dma_start(out=xb[:], in_=xtb_d[b])
                transform_block(0, b, xb)
                if b + 1 == CHUNK_BOUNDS[ci + 1]:
                    allgather_chunk(0, CHUNK_BOUNDS[ci], CHUNK_BOUNDS[ci + 1])
                    ci += 1

            aggregate(0)   # interleaves transform(1) + allgather chunks
            aggregate(1)   # interleaves transform(2) + allgather chunks
            aggregate(2)

    nc.compile()
    return nc


def _get_built():
    global _BUILT
    if _BUILT is None:
        _BUILT = build_kernel()
    return _BUILT


def kernel(**inputs) -> np.ndarray:
    from concourse.bass_utils import run_bass_kernel_spmd

    pp = preprocess(np.asarray(inputs["edge_index"]))
    in_maps = build_core_inputs(inputs, pp)
    nc = _get_built()
    res = run_bass_kernel_spmd(nc, in_maps, core_ids=list(range(NCORES)))
    parts = np.stack([r["out_part"][0] for r in res.results])  # [8, 64]
    g = parts.sum(axis=0, keepdims=True) / N
    out = (g @ np.asarray(inputs["hw"], np.float32)
           + np.asarray(inputs["hb"], np.float32)).astype(np.float32)
    return out
